# revision 10
# baseline (speedup 1.0000x reference)
"""Trainium2 Bass kernel v2 for windowed multi-head attention with relative
position bias (nn_Attention_44006234915573).

Structure per window (625 tokens, d=128, 4 heads of 32):
  qkv = x @ Wqkv^T (PE, bf16). Scores computed transposed and head-PAIR
  packed: one 3-psum-bank tile holds S^T for heads (2p, 2p+1) at cols
  [0:625] and [625:1250], so ONE ACT exp instruction and ONE DVE
  bias-multiply (vs exp(bias) tables) cover a whole pair. AV rides a
  fused ones-column in V' to produce softmax denominators Z as extra
  rows; 1/Z via DRAM-bounce broadcast + DVE reciprocal; out-projection
  is a single contraction-128 matmul pair over the merged head-major
  normalized outputs.

Engine budget per window (target): ACT ~12.9us (cap), PE ~11.7us
(@max p-state), DVE ~11us, Pool (evictions + some muls) ~8us, SP ~8us.
Data parallel over windows: 32 per core x 8 cores.
"""

import sys
import types
import contextlib
import ctypes
from contextlib import ExitStack

import numpy as np
import ml_dtypes

import bass_rust as _bass_rust
import concourse.bass as bass
import concourse.tile as tile
from concourse import mybir
from concourse.vector_clock import ScopedClock

BATCH = 256
D = 128
WS = 25
N = WS * WS  # 625
H = 4
DH = 32
SCALE = DH**-0.5
NCORES = 8
WPC = BATCH // NCORES  # 32
JC = 5  # column chunks of 125
PCH = N // JC  # 125
P2 = 2  # head pairs

BF16 = mybir.dt.bfloat16
F32 = mybir.dt.float32

# S-pair psum column layout: head A at [0:625], head B at [640:1265]
# (16-aligned start); matmul writes may not cross psum bank boundaries
HB = 640  # head-B column base in the S-pair psum tile / e tiles
EW = HB + N  # 1265
SCH = (  # (half, psum_col, q_col, length)
    (0, 0, 0, 512),
    (1, HB, 0, 384),
    (0, 512, 512, 113),
    (1, 1024, 384, 241),
)
QKCH = (  # q|k psum layout keeps the tight [0:625],[625:1250] packing
    (0, 0, 0, 512),
    (1, 625, 0, 399),
    (0, 512, 512, 113),
    (1, 1024, 399, 226),
)
AVCH = ((0, 512), (512, 113))  # i-chunks for AV / proj / qk

# engine-balance knobs (GPSIMD cannot touch PSUM, so it gets SBUF-only work):
# onorm muls on gpsimd, by (pk, half)
POOL_ONORM = frozenset({(0, 0), (0, 1), (1, 0), (1, 1)})
# bias-multiply (stage t, half) single-head ops that run on gpsimd
POOL_BIAS = frozenset({(0, 1), (2, 1), (5, 1), (7, 1)})
# Z broadcast across partitions: gpsimd partition_broadcast (needs SBUF src)
# or DRAM-bounce DMA (False)
Z_VIA_POOL = False
# PE-queue stage of window b+1 at which window b's projection is emitted
PROJ_T = 4


# ---------------------------------------------------------------------------
# workaround: this container's walrus rejects >1 sem wait on the kernel-tail
# Drain. Split the waits one-per-Drain.
def _patched_drain_and_barrier(self, tick_clock, wait_clock):
    nc = self.nc
    drain_inst = nc.sync.drain()
    wait_clock.add_sem_waits(
        drain_inst.ins, ScopedClock({None: tick_clock.global_clock})
    )
    si = drain_inst.ins.sync_info
    waits = list(si.on_wait)
    if len(waits) > 1:
        drain_inst.ins.sync_info = type(si)(on_wait=[], on_update=[])
        id2sem = {h.num: h for h in self.sems.allocated().values()}
        for w in waits:
            d = nc.sync.drain()
            _bass_rust.wait_op(d.ins, id2sem[w.id], w.wait_value, "sem-ge", False)
    nc.all_engine_barrier()
    popped = nc._tile_sem_poison_stack.pop()
    assert popped is self._sem_poison
    nc.clear_and_free_semaphores(list(self.sems.allocated().values()))
    nc.all_engine_barrier()


tile.TileContext._drain_and_barrier = _patched_drain_and_barrier


def _split_multi_waits(nc):
    """This walrus build accepts at most ONE sem wait per instruction; Tile's
    wait assignment can attach several. Move extras onto preceding nops on the
    same engine."""
    scratch_bb = nc.cur_bb.bb if nc.cur_bb is not None else None
    for f in nc.m.functions:
        for bb in f.blocks:
            lst = bb.instructions
            i = 0
            while i < len(lst):
                inst = lst[i]
                si = getattr(inst, "sync_info", None)
                if si is None:
                    i += 1
                    continue
                waits = list(si.on_wait)
                if len(waits) <= 1:
                    i += 1
                    continue
                SyncInfo = type(si)
                inst.sync_info = SyncInfo(
                    on_wait=[waits[-1]], on_update=list(si.on_update)
                )
                eng = nc.engines[inst.engine]
                for w in waits[:-1]:
                    nop = eng.nop(nofuse=True).ins
                    nop.sync_info = SyncInfo(on_wait=[w], on_update=[])
                    # eng.nop() appended to the current bb; move it here
                    for blk in f.blocks:
                        l2 = blk.instructions
                        if l2 and l2[-1] is nop:
                            l2.pop()
                            break
                    else:
                        if scratch_bb is not None:
                            l2 = scratch_bb.instructions
                            if l2 and l2[-1] is nop:
                                l2.pop()
                    lst.insert(i, nop)
                    i += 1
                i += 1


# ---------------------------------------------------------------------------
# NTFF profiling hook (only exercised when trace=True): the RL image's antenv
# lacks axon_hooks; install the ctypes equivalent of trn_boot's hook.
def _install_ntff_hook():
    if "antenv.axon_hooks" in sys.modules:
        return
    so_path = "/opt/axon/libaxon_pjrt.so"
    try:
        lib = ctypes.CDLL(so_path)
    except OSError:
        return
    if not hasattr(lib, "axon_start_nrt_profile"):
        return
    lib.axon_start_nrt_profile.argtypes = [
        ctypes.POINTER(ctypes.c_int64),
        ctypes.c_size_t,
    ]
    lib.axon_start_nrt_profile.restype = ctypes.c_int64
    lib.axon_stop_nrt_profile.argtypes = [ctypes.c_char_p]
    lib.axon_stop_nrt_profile.restype = ctypes.c_int64

    @contextlib.contextmanager
    def _hook(output_dir, device_ids=None):
        import jax

        jax.devices()
        if device_ids:
            ids = (ctypes.c_int64 * len(device_ids))(*device_ids)
            rc = lib.axon_start_nrt_profile(ids, len(device_ids))
        else:
            rc = lib.axon_start_nrt_profile(None, 0)
        if rc != 0:
            raise RuntimeError(f"axon_start_nrt_profile rc={rc}")
        try:
            yield
        finally:
            n = lib.axon_stop_nrt_profile(str(output_dir).encode())
            print(f"profile: {n} file(s) -> {output_dir}", file=sys.stderr)

    mod = types.ModuleType("antenv.axon_hooks")
    mod._hook = _hook
    mod.set_axon_ntff_profile_hook = lambda h: setattr(mod, "_hook", h)
    mod.get_axon_ntff_profile_hook = lambda: mod._hook
    sys.modules["antenv.axon_hooks"] = mod
    import antenv

    antenv.axon_hooks = mod


# ---------------------------------------------------------------------------
def build_nc(wpc=WPC, sim_safe=False, pool_onorm=POOL_ONORM,
             pool_bias=POOL_BIAS, z_via_pool=Z_VIA_POOL, proj_t=PROJ_T,
             stages=5, exp_split=False, no_pos=False):
    nc = bass.Bass(target_bir_lowering=False, debug=False)

    x_d = nc.dram_tensor("x", [wpc, D, N], BF16, kind="ExternalInput")
    wqk_d = nc.dram_tensor("wqk", [D, 2 * D], BF16, kind="ExternalInput")
    wv_d = nc.dram_tensor("wv", [D, D], BF16, kind="ExternalInput")
    # per head-pair W_out^T block: rows {0:32, 64:96} hold the pair's two
    # heads' contraction rows, rows 32:64 are ZERO (they meet garbage rows
    # of the onorm tiles)
    wo_d = nc.dram_tensor("wo", [P2, 96, D], BF16, kind="ExternalInput")
    expb2_d = nc.dram_tensor("expb2", [P2, JC, PCH, 2 * N], BF16,
                             kind="ExternalInput")
    y_d = nc.dram_tensor("y", [wpc, D, N], F32, kind="ExternalOutput")
    # Z rows bounced through DRAM for the cross-partition broadcast
    zs_d = nc.dram_tensor("zscratch", [4, P2, 2, N], F32)

    with tile.TileContext(nc) as tc, ExitStack() as ctx:
        persist = ctx.enter_context(tc.tile_pool(name="persist", bufs=1))
        xpool = ctx.enter_context(tc.tile_pool(name="xpool", bufs=2))
        qkpool = ctx.enter_context(tc.tile_pool(name="qkpool", bufs=2))
        epool = ctx.enter_context(tc.tile_pool(name="epool", bufs=8))
        opool = ctx.enter_context(tc.tile_pool(name="opool", bufs=3))
        rpool = ctx.enter_context(tc.tile_pool(name="rpool", bufs=6))
        ypool = ctx.enter_context(tc.tile_pool(name="ypool", bufs=2))
        # PSUM: bigps 3x2 banks + avps 1x2 banks = 8 banks
        bigps = ctx.enter_context(tc.tile_pool(name="bigps", bufs=3, space="PSUM"))
        avps = ctx.enter_context(tc.tile_pool(name="avps", bufs=1, space="PSUM"))

        # --- persistent loads ------------------------------------------------
        wqk_sb = persist.tile([D, 2 * D], BF16, tag="wqk")
        nc.sync.dma_start(wqk_sb[:, :], wqk_d[:, :])
        wv_sb = persist.tile([D, D], BF16, tag="wv")
        nc.sync.dma_start(wv_sb[:, :], wv_d[:, :])
        wo_sb = []
        for p in range(P2):
            t = persist.tile([96, D], BF16, tag=f"wo{p}")
            nc.sync.dma_start(t[:, :], wo_d[p, :, :])
            wo_sb.append(t)

        btab = {}
        for p in range(P2):
            for jc in range(JC):
                for half in range(2):
                    t = persist.tile([PCH, N], BF16,
                                     name=f"btab{p}_{jc}_{half}",
                                     tag=f"btab{p}_{jc}_{half}")
                    nc.sync.dma_start(
                        t[:, :], expb2_d[p, jc, :, half * N : (half + 1) * N]
                    )
                    btab[(p, jc, half)] = t

        # V' (n-major V with fused ones columns), double-buffered by window
        # parity to decouple window b+1's V eviction from window b's AV reads
        vprimes = []
        for v in range(2):
            t = persist.tile([PCH, JC * H * (DH + 1)], BF16, tag=f"vprime{v}")
            nc.vector.memset(t[:, :], 1.0)  # ones columns persist
            vprimes.append(t)

        # persistent per-(parity, pk) normalized-output tiles; rows {0:32,
        # 64:96} are written each window, rows 32:64 stay zero so the proj
        # matmul can contract all 96 rows against the zero-padded wo block
        onorms = [[persist.tile([96, N], BF16, name=f"on{v}_{p}",
                                tag=f"on{v}_{p}")
                   for p in range(P2)] for v in range(2)]
        for v in range(2):
            for p in range(P2):
                nc.vector.memset(onorms[v][p][32:64, :], 0.0)

        def vp(vt, jc, h):
            o = jc * H * (DH + 1) + h * (DH + 1)
            return vt[:, o : o + DH + 1]

        # --- per-window pipeline ---------------------------------------------
        xtiles = {}

        def load_x(b):
            if b < wpc and b not in xtiles:
                t = xpool.tile([D, N], BF16, tag="xb")
                nc.sync.dma_start(t[:, :], x_d[b, :, :])
                xtiles[b] = t

        pending_proj = None
        load_x(0)

        for b in range(wpc):
            load_x(b + 1)
            xb = xtiles.pop(b)
            vprime = vprimes[b % 2]

            # q^T | k^T -> (128, 1250) bf16, one 2-bank slot per part
            qk = qkpool.tile([D, 2 * N], BF16, tag="qk")
            for part in range(2):
                ps = bigps.tile([D, 1024], F32, tag="big")
                for off, ln in AVCH:
                    nc.tensor.matmul(
                        ps[:, off : off + ln],
                        lhsT=wqk_sb[:, part * D : (part + 1) * D],
                        rhs=xb[:, off : off + ln],
                        start=True,
                        stop=True,
                    )
                nc.vector.tensor_copy(
                    qk[:, part * N : (part + 1) * N], ps[:, :N]
                )

            # V chunks: 5 matmuls into one psum, one strided copy out
            ps = bigps.tile([D, 1024], F32, tag="big")
            for jc in range(JC):
                nc.tensor.matmul(
                    ps[:PCH, jc * D : (jc + 1) * D],
                    lhsT=xb[:, jc * PCH : (jc + 1) * PCH],
                    rhs=wv_sb[:, :],
                    start=True,
                    stop=True,
                )
            vdst = vprime[:, :].rearrange(
                "p (j g c) -> p j g c", j=JC, g=H
            )[:, :, :, 0:DH]
            vsrc = ps[:PCH, : JC * D].rearrange("p (j g c) -> p j g c", j=JC, g=H)
            nc.vector.tensor_copy(vdst, vsrc)

            if stages < 2 and stages not in (11, 12):
                ysb = ypool.tile([D, N], F32, tag="ysb")
                nc.vector.tensor_copy(ysb[:, :], qk[:, :N])
                nc.sync.dma_start(y_d[b, :, :], ysb[:, :])
                continue

            # 10 S-pair stages, AV trailing one stage behind on the PE queue
            av = None
            osbs = {}
            onorm = onorms[b % 2]
            stage_e = {}

            def emit_S(t):
                p, jc = divmod(t, JC)
                es = []
                for half in range(2):
                    h = 2 * p + half
                    sps = bigps.tile([D, 1024], F32, tag="big")
                    kw = {} if no_pos else {"tile_position": (DH * h, 0)}
                    for off, ln in AVCH:
                        nc.tensor.matmul(
                            sps[:PCH, off : off + ln],
                            lhsT=qk[
                                DH * h : DH * (h + 1),
                                N + jc * PCH : N + (jc + 1) * PCH,
                            ],
                            rhs=qk[DH * h : DH * (h + 1), off : off + ln],
                            start=True,
                            stop=True,
                            **kw,
                        )
                    if stages == 11:
                        d = epool.tile([PCH, N], BF16, tag="e")
                        nc.vector.tensor_copy(d[:, :], sps[:PCH, :N])
                        es.append(d)
                        continue
                    e0 = epool.tile([PCH, N], BF16, tag="e")
                    nc.scalar.activation(
                        e0[:, :], sps[:PCH, :N],
                        mybir.ActivationFunctionType.Exp,
                    )
                    if stages == 12:
                        es.append(e0)
                        continue
                    e = epool.tile([PCH, N], BF16, tag="e")
                    eng = nc.gpsimd if (t, half) in pool_bias else nc.vector
                    eng.tensor_mul(e[:, :], e0[:, :], btab[(p, jc, half)][:, :])
                    es.append(e)
                stage_e[t] = es

            def emit_AV_stub(t):
                es = stage_e.pop(t)
                if t == 2 * JC - 1:
                    ysb = ypool.tile([D, N], F32, tag="ysb")
                    nc.vector.tensor_copy(ysb[:PCH, :], es[0][:, :N])
                    nc.sync.dma_start(y_d[b, :, :], ysb[:, :])

            def emit_AV(t):
                nonlocal av
                p, jc = divmod(t, JC)
                if jc == 0:
                    av = avps.tile([D, 640], F32, tag="av")
                es = stage_e.pop(t)
                for off, ln in AVCH:
                    for half, rowbase in ((0, 0), (1, 64)):
                        h = 2 * p + half
                        nc.tensor.matmul(
                            av[rowbase : rowbase + DH + 1, off : off + ln],
                            lhsT=vp(vprime, jc, h),
                            rhs=es[half][:, off : off + ln],
                            start=(jc == 0),
                            stop=(jc == JC - 1),
                            tile_position=(0, rowbase),
                            skip_group_check=True,
                        )
                if jc == JC - 1:
                    finish_pair(p)

            def finish_pair(pk):
                # O' + Z rows out of PSUM (releases av for the next pair)
                osb = opool.tile([D, N], F32, tag="osb")
                if sim_safe:
                    nc.vector.tensor_copy(osb[:33, :], av[:33, :N])
                    nc.vector.tensor_copy(osb[64:97, :], av[64:97, :N])
                else:
                    nc.vector.tensor_copy(osb[:97, :], av[:97, :N])
                osbs[pk] = osb
                if stages < 4:
                    if pk == 1:
                        ysb = ypool.tile([D, N], F32, tag="ysb")
                        nc.vector.tensor_copy(ysb[:, :], osb[:, :])
                        nc.sync.dma_start(y_d[b, :, :], ysb[:, :])
                    return
                R = rpool.tile([D, N], F32, tag="R")
                if z_via_pool:
                    # broadcast Z rows across partitions (SBUF src only;
                    # needs library_config with InstPartitionBroadcast)
                    nc.gpsimd.partition_broadcast(R[0:DH, :], osb[32:33, :])
                    nc.gpsimd.partition_broadcast(R[64:96, :], osb[96:97, :])
                else:
                    # Z rows -> DRAM -> broadcast-load
                    zd = zs_d[b % 4, pk]
                    nc.sync.dma_start(zd[0, :], osb[32:33, :])
                    nc.sync.dma_start(zd[1, :], osb[96:97, :])
                    for a, r0 in ((0, 0), (1, 64)):
                        zap = zd[a, :]
                        bsrc = bass.AP(
                            zap.tensor, zap.offset, [[0, DH], [1, N]]
                        )
                        nc.sync.dma_start(R[r0 : r0 + DH, :], bsrc)
                # one reciprocal over rows 0:96 (garbage middle never read);
                # all operands on the same partition range for the verifier
                R2 = rpool.tile([D, N], F32, tag="R2")
                nc.vector.reciprocal(R2[0:32, :], R[0:32, :])
                nc.vector.reciprocal(R2[64:96, :], R[64:96, :])
                # normalize in place in head rows {0:32, 64:96}
                for half, r0 in ((0, 0), (1, 64)):
                    eng = nc.gpsimd if (pk, half) in pool_onorm else nc.vector
                    eng.tensor_mul(
                        onorm[pk][r0 : r0 + DH, :],
                        osb[r0 : r0 + DH, :],
                        R2[r0 : r0 + DH, :],
                    )

            def make_proj(onorm_b, b_):
                def proj():
                    pps = bigps.tile([D, 1024], F32, tag="big")
                    for off, ln in AVCH:
                        for p in range(P2):
                            nc.tensor.matmul(
                                pps[:, off : off + ln],
                                lhsT=wo_sb[p][:, :],
                                rhs=onorm_b[p][:, off : off + ln],
                                start=(p == 0),
                                stop=(p == P2 - 1),
                            )
                    ysb = ypool.tile([D, N], F32, tag="ysb")
                    nc.vector.tensor_copy(ysb[:, :], pps[:, :N])
                    nc.sync.dma_start(y_d[b_, :, :], ysb[:, :])

                return proj

            av_fn = emit_AV if stages >= 3 else emit_AV_stub
            if stages in (11, 12):
                av_fn = emit_AV_stub
            for t in range(2 * JC):
                emit_S(t)
                if t == proj_t and pending_proj is not None:
                    pending_proj()
                    pending_proj = None
                if t > 0:
                    av_fn(t - 1)
            av_fn(2 * JC - 1)

            if stages >= 5:
                pending_proj = make_proj(onorm, b)
            elif stages == 4:
                osb1 = osbs[1]
                ysb = ypool.tile([D, N], F32, tag="ysb")
                nc.vector.tensor_copy(ysb[:, :], osb1[:, :])
                nc.sync.dma_start(y_d[b, :, :], ysb[:, :])

        if pending_proj is not None:
            pending_proj()

    _split_multi_waits(nc)
    return nc


# ---------------------------------------------------------------------------
def host_prep(x, W_qkv, W_out, bias_table, rel_pos_indices):
    """Precompute the replicated device inputs (numpy, bf16)."""
    x = np.asarray(x, np.float32)
    W_qkv = np.asarray(W_qkv, np.float32)
    W_out = np.asarray(W_out, np.float32)
    bias_table = np.asarray(bias_table, np.float32)
    idx = np.asarray(rel_pos_indices)

    bf = ml_dtypes.bfloat16
    xb = x.reshape(BATCH, D, N).astype(bf)

    Wq = W_qkv[0:D] * SCALE
    Wk = W_qkv[D : 2 * D]
    Wv = W_qkv[2 * D : 3 * D]
    wqk = np.concatenate([Wq.T, Wk.T], axis=1).astype(bf)  # (128, 256)
    wv = Wv.T.astype(bf)  # (128, 128)
    WoT = W_out.T  # (c, dout), c head-major
    wo = np.zeros((P2, 96, D), np.float32)
    for p in range(P2):
        wo[p, 0:DH] = WoT[2 * p * DH : (2 * p + 1) * DH]
        wo[p, 64:96] = WoT[(2 * p + 1) * DH : (2 * p + 2) * DH]
    wo = wo.astype(bf)

    # bias^T per head: biast[h, j, i] = bias_table[idx[i, j], h]
    bfull = bias_table[idx]  # (i, j, H)
    biast = np.ascontiguousarray(np.transpose(bfull, (2, 1, 0)))  # (H, j, i)
    expb = np.exp(biast)  # (H, N, N)
    expb2 = np.empty((P2, JC, PCH, 2 * N), np.float32)
    for p in range(P2):
        for jc in range(JC):
            expb2[p, jc, :, 0:N] = expb[2 * p, jc * PCH : (jc + 1) * PCH, :]
            expb2[p, jc, :, N : 2 * N] = expb[
                2 * p + 1, jc * PCH : (jc + 1) * PCH, :
            ]
    return {
        "x": xb,
        "wqk": wqk,
        "wv": wv,
        "wo": wo,
        "expb2": expb2.astype(bf),
    }


_NC_CACHE = {}


def _get_nc(wpc):
    import os

    stages = int(os.environ.get("K2_STAGES", "5"))
    exp_split = int(os.environ.get("K2_EXPSPLIT", "0"))
    no_pos = bool(int(os.environ.get("K2_NOPOS", "0")))
    key = (wpc, stages, exp_split, no_pos)
    if key not in _NC_CACHE:
        _NC_CACHE[key] = build_nc(wpc, stages=stages, exp_split=exp_split,
                                  no_pos=no_pos)
    return _NC_CACHE[key]


def run(inputs, trace=False, wpc=WPC):
    """Run on 8 NeuronCores; returns (out, BassKernelResults)."""
    from concourse.bass_utils import run_bass_kernel_spmd

    if trace:
        _install_ntff_hook()
    prep = host_prep(
        inputs["x"], inputs["W_qkv"], inputs["W_out"],
        inputs["bias_table"], inputs["rel_pos_indices"],
    )
    shared = {k: v for k, v in prep.items() if k != "x"}
    xb = prep["x"]
    in_maps = [
        {"x": xb[i * wpc : (i + 1) * wpc], **shared} for i in range(NCORES)
    ]
    nc = _get_nc(wpc)
    res = run_bass_kernel_spmd(nc, in_maps, list(range(NCORES)), trace=trace)
    out = np.concatenate([res.results[i]["y"] for i in range(NCORES)], axis=0)
    out = out.reshape(BATCH, D, WS, WS).astype(np.float32)
    return out, res


def kernel(x, W_qkv, W_out, bias_table, rel_pos_indices):
    out, _ = run(
        {
            "x": x,
            "W_qkv": W_qkv,
            "W_out": W_out,
            "bias_table": bias_table,
            "rel_pos_indices": rel_pos_indices,
        },
        trace=False,
    )
    return out
